# revision 8
# baseline (speedup 1.0000x reference)
"""BertSelfAttention on 8 Trainium2 NeuronCores (Bass/Tile).

Sharding: data-parallel over batch (B=2) x tensor-parallel over heads
(16 heads -> 4 groups of 4). Core c handles batch c//4, head group c%4,
holding column shards of Wq/Wk/Wv. No collectives.

v3: TE-dense schedule. At sustained PE clock the TensorE row count
(~300k cycles ~ 127us @2.4GHz) is the hard floor; everything else must
stay off the critical path:

  * exp split per score tile: ScalarE does one 512-col half (exact exp),
    DVE does the other via a Schraudolph bit-trick in ONE tensor_scalar:
    i16 = round(s * 0.125*128*log2e + (127-sigma)*128) bitcast as bf16
    (~3% max rel err on half the probability mass -> ~1.1e-2 total rel
    err, inside the 2e-2 gate). Each half ~0.7us, so the per-step exp
    wall drops from 1.1-1.3us to ~0.7us and no longer paces the loop.
  * streamed head: xT arrives block-major ([128, 4 blk, 8 kt, 512]; one
    contiguous 8KB/partition DMA per block) and the m=0 K/Q projections
    run per 512-seq block as soon as its DMA lands; pass (0,0) starts
    ~9us in instead of ~28us. GPSIMD cannot touch PSUM (BIR verifier)
    so evacuations stay on ACT (K-m0, ctx out) and DVE (Q-m0, V, m1).
  * fillers (remaining m0 blocks, V tiles, full-sweep m1 units) are
    deadline-scheduled into the per-step TE slack so the PE never idles
    (idle gaps also drop the PE p-state, halving matmul rate).

Engine budget per core: TE ~127us (bound), DVE ~106us, ACT ~102us.

PSUM (8 banks):
  tag "ssc" 2x[128,1024] (4): score tiles, double buffered
  tag "ctx" 2x[65,512]   (2): ctx+denominator accumulators (hh pair)
  tag "a"   2x[128,512]  (2): m0 blocks / V-proj / m1 sweeps / warmup

Per head the ctx stationary is [V_h | ones] (65 cols): PSUM row 65 of
each ctx tile accumulates the softmax denominators for free. Host
unshards: out[b, :, g*256 + 64h + r] = (ctx_h / sums_h).T
"""

import sys

sys.path.insert(0, "/opt/trn_rl_repo")

import numpy as np

try:
    import ml_dtypes

    _BF16 = ml_dtypes.bfloat16
except ImportError:  # pragma: no cover
    import jax.numpy as jnp

    _BF16 = jnp.bfloat16

import concourse.bass as bass
import concourse.mybir as mybir
import concourse.tile as tile
from concourse import bacc
from concourse import bass_utils as _bass_utils
from concourse.bass_utils import run_bass_kernel_spmd

F32 = mybir.dt.float32
BF16 = mybir.dt.bfloat16
I16 = mybir.dt.int16

HIDDEN = 1024
NUM_HEADS = 16
HEAD = 64
B, S = 2, 2048
N_CORES = 8
GROUPS = 4                      # head groups (tensor parallel)
HG = NUM_HEADS // GROUPS        # heads per group = 4
DG = HG * HEAD                  # 256 cols per group
KT_TILES = HIDDEN // 128        # 8 contraction tiles for projections
ST_TILES = S // 128             # 16 sequence tiles
QC = 512                        # q chunk width (one pass = one chunk)
N_QC = S // QC                  # 4
NBLK = 4                        # xT streaming blocks of 512 seq positions
VAUG = HG * (HEAD + 1)          # 260: [V_h | ones] per head

# Schraudolph fast-exp constants (bf16 exponent domain, minimax sigma).
# es = bitcast_bf16(int16(round(s * EXP_MUL + EXP_ADD))) ~= exp(s / 8)
_LOG2E = 1.4426950408889634
EXP_MUL = 0.125 * 128.0 * _LOG2E
EXP_ADD = (127.0 - 0.04303) * 128.0


def _build_kernel():
    nc = bacc.Bacc("TRN2")

    # xT block-major: xTb[p, b, kt, s] = x[b*512+s, kt*128+p]; each
    # [:, b] slice is 8KB contiguous per partition on both sides.
    xTb = nc.dram_tensor("xTb", [128, NBLK, KT_TILES, QC], BF16,
                         kind="ExternalInput")
    # wqk[p, m, kt, :] = [Wq_m | Wk_m][kt*128+p, :] (partition-major
    # SBUF image; 4KB per-partition DMA segments).
    wqk = nc.dram_tensor(
        "wqk", [128, 2, KT_TILES, DG], BF16, kind="ExternalInput"
    )
    # wv pre-augmented (per head 64 cols + zero col), partition-major.
    wv = nc.dram_tensor(
        "wv", [128, KT_TILES, VAUG], BF16, kind="ExternalInput"
    )
    # per-partition bias cols: bq[0:128], bq[128:], bk[0:128], bk[128:]
    bqk = nc.dram_tensor("bqk", [128, 4], F32, kind="ExternalInput")
    # bv interleaved with 1.0 at each head's ones column [1, 260]
    bv_aug = nc.dram_tensor("bv_aug", [1, VAUG], BF16, kind="ExternalInput")
    out_raw = nc.dram_tensor("out_raw", [VAUG, S], F32, kind="ExternalOutput")

    with tile.TileContext(nc) as tc:
        with (
            tc.tile_pool(name="consts", bufs=1) as consts,
            tc.tile_pool(name="esp", bufs=3) as esp,
            tc.tile_pool(name="outp", bufs=4) as outp,
            tc.tile_pool(name="ps", bufs=2, space="PSUM") as ps,
        ):
            # ---- loads. A single DGE queue sustains only ~140 GB/s, so
            # the critical tensors are spread across four engine queues
            # (each SBUF tile written by exactly one queue — two queues
            # on one tile wedges the device). Block 0 rides the
            # otherwise-idle Pool queue so pass (0,0) can start earliest.
            # wqk m0 split across two queues (halves are separate tiles)
            wqk0a_sb = consts.tile([128, 4, DG], BF16)   # kt 0..3
            wqk0b_sb = consts.tile([128, 4, DG], BF16)   # kt 4..7
            nc.sync.dma_start(wqk0a_sb[:], wqk[:, 0, 0:4])
            nc.scalar.dma_start(wqk0b_sb[:], wqk[:, 0, 4:8])

            def wqk0_kt(kt):
                return (wqk0a_sb[:, kt] if kt < 4 else wqk0b_sb[:, kt - 4])

            bqk_sb = consts.tile([128, 4], F32)
            nc.scalar.dma_start(bqk_sb[:], bqk[:])
            bvaug_sb = consts.tile([1, VAUG], BF16)
            nc.scalar.dma_start(bvaug_sb[:], bv_aug[:])
            ones_sb = consts.tile([1, QC], BF16)
            nc.vector.memset(ones_sb[:], 1.0)

            wqk1_sb = consts.tile([128, KT_TILES, DG], BF16)
            wv_sb = consts.tile([128, KT_TILES, VAUG], BF16)
            xTs = [consts.tile([128, KT_TILES, QC], BF16, name=f"xt{b}")
                   for b in range(NBLK)]
            # gpsimd queue: block 0 first (earliest consumer), then wv
            # (V0 at ~step 0) and wqk1 (m1 fillers from ~step 16).
            nc.gpsimd.dma_start(xTs[0][:], xTb[:, 0])
            nc.gpsimd.dma_start(wv_sb[:], wv[:])
            nc.gpsimd.dma_start(wqk1_sb[:], wqk[:, 1])
            nc.sync.dma_start(xTs[1][:], xTb[:, 1])
            nc.scalar.dma_start(xTs[2][:], xTb[:, 2])
            nc.sync.dma_start(xTs[3][:], xTb[:, 3])

            QT_sb = consts.tile([128, 2, S], BF16)
            KT_sb = consts.tile([128, 2, S], BF16)
            v_sb = consts.tile([128, ST_TILES, VAUG], BF16)

            # Filler units are (matmul-emit, evac-emit) pairs. Within a
            # step the emission order is: filler MMs, scores+exp,
            # ctx(i-1), filler evacs. Engine queues are in-order, so an
            # evac whose producing matmuls haven't retired would
            # head-of-line block the latency-critical exp op behind it;
            # with MMs at the head of the same step's TE queue, the evac
            # deps are long satisfied by the time the evac is queued.
            _acc = {}

            # ---- m=0 projection for one 512-seq block ----
            # wqk m0 layout per kt: Q cols 0:128, K cols 128:256.
            def m0_mm(b, which, warm=False):
                col0 = 0 if which == "q" else 128
                acc = ps.tile([128, QC], F32, tag="a", name=f"m0{which}{b}")
                _acc[("m0", which, b)] = acc
                if warm:
                    # p-state warm-up: hold the PE busy through the DMA
                    # load phase; garbage erased by kt=0's start=True.
                    for _ in range(9):
                        nc.tensor.matmul(
                            acc[:], ones_sb[0:1, 0:128], ones_sb[0:1, :],
                            start=True, stop=True,
                        )
                for kt in range(KT_TILES):
                    nc.tensor.matmul(
                        acc[:],
                        wqk0_kt(kt)[:, col0:col0 + 128],
                        xTs[b][:, kt, :],
                        start=(kt == 0), stop=(kt == KT_TILES - 1),
                    )

            def m0_evac(b, which):
                acc = _acc.pop(("m0", which, b))
                if which == "k":
                    nc.scalar.activation(
                        KT_sb[:, 0, b * QC:(b + 1) * QC], acc[:],
                        mybir.ActivationFunctionType.Identity,
                        bias=bqk_sb[:, 2:3],
                    )
                else:
                    nc.vector.tensor_scalar_add(
                        QT_sb[:, 0, b * QC:(b + 1) * QC], acc[:],
                        bqk_sb[:, 0:1],
                    )

            def v_mm(st):
                psv = ps.tile([128, QC], F32, tag="a", name="psv")
                _acc[("v", st)] = psv
                blk, sub = st // 4, st % 4
                for kt in range(KT_TILES):
                    nc.tensor.matmul(
                        psv[:, 0:VAUG],
                        xTs[blk][:, kt, sub * 128:(sub + 1) * 128],
                        wv_sb[:, kt, :],
                        start=(kt == 0), stop=False,
                    )
                nc.tensor.matmul(
                    psv[:, 0:VAUG], ones_sb[:, 0:128], bvaug_sb[:, :],
                    start=False, stop=True,
                )

            def v_evac(st):
                psv = _acc.pop(("v", st))
                nc.vector.tensor_copy(out=v_sb[:, st, :], in_=psv[:, 0:VAUG])

            # m=1 projection: one full 8-kt sweep per unit.
            def m1_mm(wcol, sc):
                acc = ps.tile([128, QC], F32, tag="a",
                              name=f"m1_{wcol}_{sc}")
                _acc[("m1", wcol, sc)] = acc
                for kt in range(KT_TILES):
                    nc.tensor.matmul(
                        acc[:],
                        wqk1_sb[:, kt, wcol:wcol + 128],
                        xTs[sc][:, kt, :],
                        start=(kt == 0), stop=(kt == KT_TILES - 1),
                    )

            def m1_evac(dst_sb, wcol, bcol, sc):
                acc = _acc.pop(("m1", wcol, sc))
                nc.vector.tensor_scalar_add(
                    dst_sb[:, 1, sc * QC:(sc + 1) * QC], acc[:],
                    bqk_sb[:, bcol:bcol + 1],
                )

            # ---- head: warmup + block-0 m0 + V0/V1 matmuls (their
            # evacs are step-0 filler work: ACT/DVE are idle here).
            m0_mm(0, "k", warm=True)
            m0_evac(0, "k")
            m0_mm(0, "q")
            m0_evac(0, "q")
            v_mm(0)
            v_mm(1)

            # ---- filler schedule: {step: [(mm_fn, evac_fn), ...]} ----
            # At most 2 units per step (tag "a" has 2 slots; a unit's
            # psum lives only phase1->phase4 of its step).
            F = {}

            def unit(step, mm, evac):
                F.setdefault(step, []).append((mm, evac))

            unit(0, None, lambda: v_evac(0))
            unit(0, None, lambda: v_evac(1))
            # K-m0 block b: scores of pass (0,0) hit kt=4b at step 4b.
            unit(1, lambda: m0_mm(1, "k"), lambda: m0_evac(1, "k"))
            unit(2, lambda: v_mm(2), lambda: v_evac(2))
            unit(2, lambda: v_mm(3), lambda: v_evac(3))
            unit(3, lambda: v_mm(4), lambda: v_evac(4))
            unit(4, lambda: m0_mm(2, "k"), lambda: m0_evac(2, "k"))
            unit(5, lambda: v_mm(5), lambda: v_evac(5))
            unit(6, lambda: v_mm(6), lambda: v_evac(6))
            unit(7, lambda: m0_mm(3, "k"), lambda: m0_evac(3, "k"))
            for st in range(7, ST_TILES):
                unit(st, lambda st=st: v_mm(st), lambda st=st: v_evac(st))
            # Q-m0 block j feeds pass (0,j) starting at step 16j.
            unit(10, lambda: m0_mm(1, "q"), lambda: m0_evac(1, "q"))
            unit(26, lambda: m0_mm(2, "q"), lambda: m0_evac(2, "q"))
            unit(42, lambda: m0_mm(3, "q"), lambda: m0_evac(3, "q"))
            # m=1 sweeps: K all before pass (1,0) at step 64; Q chunk sc
            # before pass (1,sc) at step 64+16sc.
            for i, dl in enumerate((16, 24, 34, 44)):
                unit(dl, lambda s=i: m1_mm(128, s),
                     lambda s=i: m1_evac(KT_sb, 128, 3, s))
            for i, dl in enumerate((54, 70, 86, 100)):
                unit(dl, lambda s=i: m1_mm(0, s),
                     lambda s=i: m1_evac(QT_sb, 0, 1, s))

            # ---- attention: 8 passes x 16 kt steps, lag-1 pipeline ----
            passes = [(p, j) for p in range(2) for j in range(N_QC)]
            steps = [(pi, kt) for pi in range(len(passes))
                     for kt in range(ST_TILES)]
            n_steps = len(steps)

            es_tiles = {}
            ctx_tiles = {}

            def emit_scores(i):
                pi, kt = steps[i]
                p, j = passes[pi]
                ssc = ps.tile([128, 2 * QC], F32, tag="ssc", name="ssc")
                for hh in range(2):
                    rows = slice(hh * 64, hh * 64 + 64)
                    nc.tensor.matmul(
                        ssc[:, hh * QC:(hh + 1) * QC],
                        KT_sb[rows, p, kt * 128:(kt + 1) * 128],
                        QT_sb[rows, p, j * QC:(j + 1) * QC],
                        start=True, stop=True,
                    )
                es = esp.tile([128, 2 * QC], BF16, tag="es", name="es")
                hh_act = i % 2          # exact-exp half alternates
                hh_dve = 1 - hh_act
                sa = slice(hh_act * QC, (hh_act + 1) * QC)
                sd = slice(hh_dve * QC, (hh_dve + 1) * QC)
                nc.scalar.activation(
                    es[:, sa], ssc[:, sa],
                    mybir.ActivationFunctionType.Exp, scale=0.125,
                )
                nc.vector.tensor_scalar(
                    es[:, sd].bitcast(I16), ssc[:, sd],
                    EXP_MUL, EXP_ADD,
                    mybir.AluOpType.mult, mybir.AluOpType.add,
                )
                es_tiles[i] = es

            pending_out = []

            def emit_ctx(i):
                pi, kt = steps[i]
                p, j = passes[pi]
                if kt == 0:
                    for hh in range(2):
                        ctx_tiles[(pi, hh)] = ps.tile(
                            [65, QC], F32, tag="ctx", name="ctx"
                        )
                es = es_tiles.pop(i)
                for hh in range(2):
                    h = 2 * p + hh
                    nc.tensor.matmul(
                        ctx_tiles[(pi, hh)][:],
                        v_sb[:, kt, h * 65:(h + 1) * 65],
                        es[:, hh * QC:(hh + 1) * QC],
                        start=(kt == 0), stop=(kt == ST_TILES - 1),
                    )
                if kt == ST_TILES - 1:
                    pending_out.append((pi,
                                        ctx_tiles.pop((pi, 0)),
                                        ctx_tiles.pop((pi, 1))))

            def flush_out():
                # pass-boundary out evac: hh0 on ACT, hh1 on DVE, run in
                # parallel; emitted at the START of the following step so
                # the ctx-stop matmul has retired and the copies don't
                # stall the exp ops queued after them.
                while pending_out:
                    pi, c0, c1 = pending_out.pop(0)
                    p, j = passes[pi]
                    for hh, cpsum in ((0, c0), (1, c1)):
                        h = 2 * p + hh
                        ctx_sb = outp.tile([65, QC], F32, tag="o",
                                           name="ctx_sb")
                        if hh == 0:
                            nc.scalar.copy(out=ctx_sb[:], in_=cpsum[:])
                        else:
                            nc.vector.tensor_copy(out=ctx_sb[:],
                                                  in_=cpsum[:])
                        nc.sync.dma_start(
                            out_raw[h * 65:(h + 1) * 65,
                                    j * QC:(j + 1) * QC],
                            ctx_sb[:],
                        )

            for i in range(n_steps):
                units = F.get(i, ())
                flush_out()
                for mm, _ in units:          # phase 1: filler matmuls
                    if mm:
                        mm()
                emit_scores(i)               # phase 2: scores + exp
                if i > 0:
                    emit_ctx(i - 1)          # phase 3
                for _, evac in units:        # phase 4: filler evacs
                    if evac:
                        evac()
            emit_ctx(n_steps - 1)
            flush_out()
    nc.compile()
    return nc


_NC_CACHE = None


def _get_nc():
    global _NC_CACHE
    if _NC_CACHE is None:
        _NC_CACHE = _build_kernel()
    return _NC_CACHE


def _prep_core_inputs(hidden_states, Wq, bq, Wk, bk, Wv, bv):
    """Host-side sharding: returns list of 8 in_maps (bf16 pre-cast)."""
    # xT [1024, 2048] -> block-major [128 p, 4 blk, 8 kt, 512 s]
    xTbs = [
        np.ascontiguousarray(
            hidden_states[b].T.reshape(KT_TILES, 128, NBLK, QC)
            .transpose(1, 2, 0, 3)
        ).astype(_BF16)
        for b in range(B)
    ]
    in_maps = []
    for c in range(N_CORES):
        b, g = divmod(c, GROUPS)
        cs = slice(g * DG, (g + 1) * DG)
        wq_g = Wq[:, cs]
        wk_g = Wk[:, cs]
        wv_g = Wv[:, cs]
        bq_g, bk_g, bv_g = bq[cs], bk[cs], bv[cs]

        wv_aug = np.zeros((HIDDEN, VAUG), dtype=np.float32)
        bv_aug = np.zeros((1, VAUG), dtype=np.float32)
        for h in range(HG):
            wv_aug[:, h * 65:h * 65 + 64] = wv_g[:, h * 64:(h + 1) * 64]
            bv_aug[0, h * 65:h * 65 + 64] = bv_g[h * 64:(h + 1) * 64]
            bv_aug[0, h * 65 + 64] = 1.0

        bqk = np.stack(
            [bq_g[:128], bq_g[128:], bk_g[:128], bk_g[128:]], axis=1
        ).astype(np.float32)

        in_maps.append(
            {
                "xTb": xTbs[b],
                # partition-major SBUF image [128, 2, 8, 256]
                "wqk": np.ascontiguousarray(
                    np.stack([
                        np.concatenate(
                            [wq_g[:, m * 128:(m + 1) * 128],
                             wk_g[:, m * 128:(m + 1) * 128]], 1
                        ).reshape(KT_TILES, 128, DG).transpose(1, 0, 2)
                        for m in range(2)
                    ], axis=1)
                ).astype(_BF16),
                # partition-major SBUF image [128, 8, 260]
                "wv": np.ascontiguousarray(
                    wv_aug.reshape(KT_TILES, 128, VAUG).transpose(1, 0, 2)
                ).astype(_BF16),
                "bqk": np.ascontiguousarray(bqk),
                "bv_aug": bv_aug.astype(_BF16),
            }
        )
    return in_maps


def _unshard(results):
    out = np.empty((B, S, HIDDEN), dtype=np.float32)
    for c in range(N_CORES):
        b, g = divmod(c, GROUPS)
        raw = results[c]["out_raw"]  # [260, 2048]
        for h in range(HG):
            ctx = raw[h * 65:h * 65 + 64]          # [64, S]
            sums = raw[h * 65 + 64]                # [S]
            col0 = g * DG + h * HEAD
            out[b, :, col0:col0 + HEAD] = (ctx / sums).T
    return out


def kernel(**inputs):
    inputs = {k: np.asarray(v, dtype=np.float32) for k, v in inputs.items()}
    nc = _get_nc()
    in_maps = _prep_core_inputs(**inputs)
    res = run_bass_kernel_spmd(nc, in_maps, core_ids=list(range(N_CORES)))
    return _unshard(res.results)


if __name__ == "__main__":
    rng = np.random.default_rng(0)
    scale = 1.0 / np.sqrt(HIDDEN)
    ins = {
        "hidden_states": rng.standard_normal((B, S, HIDDEN), dtype=np.float32),
        "Wq": rng.standard_normal((HIDDEN, HIDDEN), dtype=np.float32) * scale,
        "bq": rng.standard_normal(HIDDEN, dtype=np.float32) * 0.01,
        "Wk": rng.standard_normal((HIDDEN, HIDDEN), dtype=np.float32) * scale,
        "bk": rng.standard_normal(HIDDEN, dtype=np.float32) * 0.01,
        "Wv": rng.standard_normal((HIDDEN, HIDDEN), dtype=np.float32) * scale,
        "bv": rng.standard_normal(HIDDEN, dtype=np.float32) * 0.01,
    }
    out = kernel(**ins)

    def ref(x, Wq, bq, Wk, bk, Wv, bv):
        q = (x @ Wq + bq).reshape(B, S, NUM_HEADS, HEAD).transpose(0, 2, 1, 3)
        k = (x @ Wk + bk).reshape(B, S, NUM_HEADS, HEAD).transpose(0, 2, 1, 3)
        v = (x @ Wv + bv).reshape(B, S, NUM_HEADS, HEAD).transpose(0, 2, 1, 3)
        s = np.einsum("bhqd,bhkd->bhqk", q, k) / np.sqrt(HEAD)
        s = s - s.max(-1, keepdims=True)
        p = np.exp(s)
        p /= p.sum(-1, keepdims=True)
        c = np.einsum("bhqk,bhkd->bhqd", p, v)
        return c.transpose(0, 2, 1, 3).reshape(B, S, HIDDEN)

    exp = ref(
        ins["hidden_states"].astype(np.float64),
        ins["Wq"].astype(np.float64), ins["bq"].astype(np.float64),
        ins["Wk"].astype(np.float64), ins["bk"].astype(np.float64),
        ins["Wv"].astype(np.float64), ins["bv"].astype(np.float64),
    )
    print("L2 rel err:", np.linalg.norm(out - exp) / np.linalg.norm(exp))
    print("max abs err:", np.abs(out - exp).max())
